# revision 1
# baseline (speedup 1.0000x reference)
"""Trainium2 Bass kernel for nn_ActorCriticSpeakerDenseQuantized.

Contract: kernel(**inputs) takes the FULL (unsharded) inputs exactly as
produced by setup_inputs() and returns the full outputs
(probs [65536,128] f32, critic [65536] f32, vq_loss scalar f32,
idx [65536] int32).

Strategy: pure data parallel over the batch across 8 NeuronCores
(8192 rows each); all weights + the 512x64 codebook replicated. The
vq_loss mean is assembled on the host from per-core partial sums.

Per-core pipeline, per 512-row chunk:
  x [512,1024] --PE transpose--> xfT (features on partitions)
  embedding tower / W_p projection as lhsT.T @ rhs matmuls with rows on
  the moving free dim; relu+bias fused on the scalar (ACT) engine.
  VQ: u[r,k] = 2*xq_r.E_k - ||E_k||^2 via a K=65 matmul (ones row folds
  the codebook norms); argmin d == argmax u found with the DVE
  max8/max_index instructions straight out of PSUM (first-occurrence tie
  semantics match jnp.argmin). d_min = ||xq||^2 - max_u, accumulated per
  partition for the loss.
  actor head: logits per 128-row group in [rows, A] layout; softmax as
  exp (ACT, with accumulated row-sum) + normalize_recip (GPSIMD).
  critic head: c2 as an M=1 matmul -> [1, rows].
"""
import sys

sys.path.insert(0, '/opt/trn_rl_repo')

import numpy as np
from contextlib import ExitStack

import concourse.bass as bass
import concourse.tile as tile
from concourse import bacc, masks, mybir
from concourse.vector_clock import ScopedClock
from concourse.bass_utils import run_bass_kernel_spmd

F32 = mybir.dt.float32
F32R = mybir.dt.float32r
AF = mybir.ActivationFunctionType

MAX_WAITS = 1

# 'f32'  : every matmul in full fp32 (max precision, slower PE)
# 'f32r' : tower in float32r, VQ distance via f32r residual splitting
CFG = {'mm': 'f32r', 'vq': 'f32r_res'}


def _split_excess_waits(nc, maxw=MAX_WAITS):
    """walrus in this toolchain rejects instructions carrying more than one
    semaphore wait. Hoist excess waits onto same-engine NoOp carriers placed
    immediately before the offending instruction (same per-engine program
    order => identical semantics)."""
    for f in nc.m.functions:
        for b in f.blocks:
            il = list(b.instructions)
            out = []
            changed = False
            for inst in il:
                si = getattr(inst, 'sync_info', None)
                waits = list(si.on_wait) if si is not None and si.on_wait else []
                if len(waits) > maxw:
                    changed = True
                    extra, keep = waits[:-maxw], waits[-maxw:]
                    for i in range(0, len(extra), maxw):
                        nop = mybir.InstNoOp(name=f"WSPLIT-{nc.next_id()}",
                                             ins=[], outs=[])
                        nop.engine = inst.engine
                        nop.sync_info = mybir.SyncInfo(on_wait=extra[i:i + maxw],
                                                       on_update=[])
                        out.append(nop)
                    si.on_wait = keep
                out.append(inst)
            if changed:
                b.instructions = out


class TC(tile.TileContext):
    """TileContext whose kernel-tail drain is split into single-wait drains
    (multi-wait sync instructions fail codegen in this walrus build)."""

    def _drain_and_barrier(self, tick_clock, wait_clock):
        drain_inst = self.nc.sync.drain()
        wait_clock.add_sem_waits(drain_inst.ins,
                                 ScopedClock({None: tick_clock.global_clock}))
        si = drain_inst.ins.sync_info
        waits = list(si.on_wait or [])
        if len(waits) > 1:
            si.on_wait = waits[:1]
            for w in waits[1:]:
                d2 = self.nc.sync.drain()
                si2 = d2.ins.sync_info
                if si2 is None:
                    d2.ins.sync_info = mybir.SyncInfo(on_wait=[w], on_update=[])
                else:
                    si2.on_wait = [w]
        self.nc.all_engine_barrier()
        popped = self.nc._tile_sem_poison_stack.pop()
        assert popped is self._sem_poison
        self.nc.clear_and_free_semaphores(list(self.sems.allocated().values()))
        self.nc.all_engine_barrier()


def build(R, cfg=None):
    cfg = cfg or {}
    mm_dt = cfg.get('mm', 'f32')
    vq_mode = cfg.get('vq', 'f32')
    assert (mm_dt, vq_mode) in (('f32', 'f32'), ('f32r', 'f32r_res'))
    assert R % 512 == 0
    NCH = R // 512
    F, H, DV, K, A = 1024, 128, 64, 512, 128

    nc = bacc.Bacc()

    x_d = nc.declare_dram_parameter('x', [R, F], F32, isOutput=False)
    We0_d = nc.declare_dram_parameter('We0', [F, H], F32, isOutput=False)
    be0_d = nc.declare_dram_parameter('be0', [H], F32, isOutput=False)
    We1_d = nc.declare_dram_parameter('We1', [H, H], F32, isOutput=False)
    be1_d = nc.declare_dram_parameter('be1', [H], F32, isOutput=False)
    We2_d = nc.declare_dram_parameter('We2', [H, H], F32, isOutput=False)
    be2_d = nc.declare_dram_parameter('be2', [H], F32, isOutput=False)
    Wp_d = nc.declare_dram_parameter('Wp', [F, DV], F32, isOutput=False)
    bp_d = nc.declare_dram_parameter('bp', [DV], F32, isOutput=False)
    E2_d = nc.declare_dram_parameter('E2', [DV + 1, K], F32, isOutput=False)
    Wa0_d = nc.declare_dram_parameter('Wa0', [H, H], F32, isOutput=False)
    ba0_d = nc.declare_dram_parameter('ba0', [H], F32, isOutput=False)
    Wa1_d = nc.declare_dram_parameter('Wa1', [H, A], F32, isOutput=False)
    Wc0_d = nc.declare_dram_parameter('Wc0', [H, H], F32, isOutput=False)
    bc0_d = nc.declare_dram_parameter('bc0', [H], F32, isOutput=False)
    Wc1_d = nc.declare_dram_parameter('Wc1', [H, H], F32, isOutput=False)
    bc1_d = nc.declare_dram_parameter('bc1', [H], F32, isOutput=False)
    Wc2_d = nc.declare_dram_parameter('Wc2', [H, 1], F32, isOutput=False)
    bc2_d = nc.declare_dram_parameter('bc2', [1], F32, isOutput=False)

    probs_d = nc.declare_dram_parameter('probs', [R, A], F32, isOutput=True)
    critic_d = nc.declare_dram_parameter('critic', [R], F32, isOutput=True)
    idx_d = nc.declare_dram_parameter('idx', [R], mybir.dt.int32, isOutput=True)
    accm_d = nc.declare_dram_parameter('accm', [128, 1], F32, isOutput=True)
    accq_d = nc.declare_dram_parameter('accq', [DV, 1], F32, isOutput=True)

    w_dt = F32 if mm_dt == 'f32' else F32R

    with TC(nc) as tc, ExitStack() as ctx:
        consts = ctx.enter_context(tc.tile_pool(name='consts', bufs=1))
        xin = ctx.enter_context(tc.tile_pool(name='xin', bufs=2))
        xft = ctx.enter_context(tc.tile_pool(name='xft', bufs=2))
        work = ctx.enter_context(tc.tile_pool(name='work', bufs=2))
        accp = ctx.enter_context(tc.tile_pool(name='accp', bufs=1))

        pt = ctx.enter_context(tc.tile_pool(name='pt', bufs=2, space='PSUM'))
        pe0 = ctx.enter_context(tc.tile_pool(name='pe0', bufs=1, space='PSUM'))
        pp = ctx.enter_context(tc.tile_pool(name='pp', bufs=1, space='PSUM'))
        pu = ctx.enter_context(tc.tile_pool(name='pu', bufs=2, space='PSUM'))
        ptw = ctx.enter_context(tc.tile_pool(name='ptw', bufs=2, space='PSUM'))

        ident = consts.tile([128, 128], F32)
        masks.make_identity(nc, ident[:])
        We0 = consts.tile([128, 8 * H], w_dt)
        nc.sync.dma_start(We0[:], We0_d[:].rearrange("(c p) m -> p c m", p=128).bitcast(w_dt))
        Wp = consts.tile([128, 8 * DV], w_dt if vq_mode != 'f32' else F32)
        pdt = Wp.dtype
        nc.sync.dma_start(Wp[:], Wp_d[:].rearrange("(c p) m -> p c m", p=128).bitcast(pdt))
        if vq_mode == 'f32r_res':
            E2f = consts.tile([DV + 1, K], F32)
            nc.sync.dma_start(E2f[:], E2_d[:])
            E2r = consts.tile([DV + 1, K], F32R)
            nc.vector.tensor_copy(E2r[:], E2f[:])
            E2lo = consts.tile([DV + 1, K], F32R)
            nc.vector.tensor_sub(E2lo[:], E2f[:], E2r[:].bitcast(F32))
        else:
            E2 = consts.tile([DV + 1, K], F32)
            nc.sync.dma_start(E2[:], E2_d[:])
        tower = {}
        for nm, wd, bd in (('e1', We1_d, be1_d), ('e2', We2_d, be2_d),
                           ('a0', Wa0_d, ba0_d), ('c0', Wc0_d, bc0_d),
                           ('c1', Wc1_d, bc1_d)):
            W = consts.tile([128, 128], w_dt, tag=f'W_{nm}')
            nc.sync.dma_start(W[:], wd[:].bitcast(w_dt))
            b = consts.tile([128, 1], F32, tag=f'b_{nm}')
            nc.sync.dma_start(b[:], bd[:].unsqueeze(1))
            tower[nm] = (W, b)
        Wa1 = consts.tile([128, A], w_dt)
        nc.sync.dma_start(Wa1[:], Wa1_d[:].bitcast(w_dt))
        Wc2 = consts.tile([128, 1], w_dt)
        nc.sync.dma_start(Wc2[:], Wc2_d[:].bitcast(w_dt))
        be0 = consts.tile([128, 1], F32)
        nc.sync.dma_start(be0[:], be0_d[:].unsqueeze(1))
        bp = consts.tile([DV, 1], F32)
        nc.sync.dma_start(bp[:], bp_d[:].unsqueeze(1))
        bc2 = consts.tile([1, 1], F32)
        nc.sync.dma_start(bc2[:], bc2_d[:].unsqueeze(1))

        acc_m = accp.tile([128, 1], F32)
        acc_q = accp.tile([DV, 1], F32)
        nc.gpsimd.memset(acc_m[:], 0.0)
        nc.gpsimd.memset(acc_q[:], 0.0)

        for c in range(NCH):
            rows = slice(c * 512, (c + 1) * 512)
            x_sb = xin.tile([128, 4096], F32, tag='x')
            nc.sync.dma_start(x_sb[:], x_d[rows, :].rearrange("(g p) f -> p g f", p=128))

            xfT = xft.tile([128, 4096], pdt, tag='xfT')
            for fc in range(8):
                ps_t = pt.tile([128, 512], F32, tag='pt')
                for g in range(4):
                    nc.tensor.transpose(
                        ps_t[:, g * 128:(g + 1) * 128],
                        x_sb[:, g * 1024 + fc * 128: g * 1024 + (fc + 1) * 128],
                        ident[:])
                if fc % 2 == 0:
                    nc.vector.tensor_copy(xfT[:, fc * 512:(fc + 1) * 512], ps_t[:])
                else:
                    nc.scalar.copy(xfT[:, fc * 512:(fc + 1) * 512], ps_t[:])

            ps_e = pe0.tile([128, 512], F32, tag='pe')
            for fc in range(8):
                nc.tensor.matmul(ps_e[:], We0[:, fc * 128:(fc + 1) * 128],
                                 xfT[:, fc * 512:(fc + 1) * 512].bitcast(We0.dtype),
                                 start=(fc == 0), stop=(fc == 7))
            embT = work.tile([128, 512], w_dt, tag='emb1')
            nc.scalar.activation(embT[:], ps_e[:], AF.Relu, bias=be0[:], scale=1.0)

            ps_p = pp.tile([DV, 512], F32, tag='pp')
            for fc in range(8):
                nc.tensor.matmul(ps_p[:], Wp[:, fc * DV:(fc + 1) * DV],
                                 xfT[:, fc * 512:(fc + 1) * 512],
                                 start=(fc == 0), stop=(fc == 7))
            xq_aug = work.tile([DV + 1, 512], F32, tag='xq')
            nc.gpsimd.memset(xq_aug[DV:DV + 1, :], 1.0)
            xq_f32_view = xq_aug[0:DV, :].bitcast(F32)
            nc.scalar.activation(xq_f32_view, ps_p[:], AF.Identity,
                                 bias=bp[:], scale=1.0)
            if vq_mode == 'f32r_res':
                xq_r = work.tile([DV + 1, 512], F32R, tag='xqr')
                nc.vector.tensor_copy(xq_r[:], xq_aug[:])
                xq_lo = work.tile([DV + 1, 512], F32R, tag='xqlo')
                nc.vector.tensor_sub(xq_lo[:], xq_aug[:], xq_r[:].bitcast(F32))

            sqs = work.tile([DV, 1], F32, tag='sqs')
            sq_scr = work.tile([DV, 512], F32, tag='sqscr')
            nc.scalar.activation(sq_scr[:], xq_f32_view, AF.Square,
                                 bias=0.0, scale=1.0, accum_out=sqs[:])
            nc.vector.tensor_add(acc_q[:], acc_q[:], sqs[:])

            for nm in ('e1', 'e2'):
                W, b = tower[nm]
                ps = ptw.tile([128, 512], F32, tag='ptw')
                nc.tensor.matmul(ps[:], W[:], embT[:].bitcast(W.dtype),
                                 start=True, stop=True)
                embT = work.tile([128, 512], w_dt, tag=nm)
                nc.scalar.activation(embT[:], ps[:], AF.Relu, bias=b[:], scale=1.0)

            idx_sb = work.tile([128, 4], mybir.dt.int32, tag='idxc')
            for g in range(4):
                ps_u = pu.tile([128, 512], F32, tag='pu')
                if vq_mode == 'f32r_res':
                    nc.tensor.matmul(ps_u[:], xq_r[:, g * 128:(g + 1) * 128],
                                     E2r[:], start=True, stop=False)
                    nc.tensor.matmul(ps_u[:], xq_r[:, g * 128:(g + 1) * 128],
                                     E2lo[:], start=False, stop=False)
                    nc.tensor.matmul(ps_u[:], xq_lo[:, g * 128:(g + 1) * 128],
                                     E2r[:], start=False, stop=True)
                else:
                    nc.tensor.matmul(ps_u[:], xq_aug[:, g * 128:(g + 1) * 128],
                                     E2[:], start=True, stop=True)
                mx = work.tile([128, 8], F32, tag='mx')
                mi = work.tile([128, 8], mybir.dt.uint32, tag='mi')
                nc.vector.max(mx[:], ps_u[:])
                nc.vector.max_index(mi[:], mx[:], ps_u[:])
                nc.vector.tensor_add(acc_m[:], acc_m[:], mx[:, 0:1])
                nc.vector.tensor_copy(idx_sb[:, g:g + 1], mi[:, 0:1])
            nc.sync.dma_start(idx_d[rows].rearrange("(g p) -> p g", p=128), idx_sb[:])

            W, b = tower['a0']
            ps = ptw.tile([128, 512], F32, tag='ptw')
            nc.tensor.matmul(ps[:], W[:], embT[:].bitcast(W.dtype), start=True, stop=True)
            aT = work.tile([128, 512], w_dt, tag='aT')
            nc.scalar.activation(aT[:], ps[:], AF.Relu, bias=b[:], scale=1.0)

            probs_sb = work.tile([128, 512], F32, tag='probs')
            ps_l = ptw.tile([128, 512], F32, tag='ptw')
            for g in range(4):
                nc.tensor.matmul(ps_l[:, g * 128:(g + 1) * 128],
                                 aT[:, g * 128:(g + 1) * 128], Wa1[:],
                                 start=True, stop=True)
            for g in range(4):
                ex = work.tile([128, 128], F32, tag='exp')
                den = work.tile([128, 1], F32, tag='den')
                nc.scalar.activation(ex[:], ps_l[:, g * 128:(g + 1) * 128], AF.Exp,
                                     bias=0.0, scale=1.0, accum_out=den[:])
                nc.gpsimd.normalize_recip(probs_sb[:, g * 128:(g + 1) * 128],
                                          ex[:], den[:])
            nc.sync.dma_start(probs_d[rows, :].rearrange("(g p) a -> p g a", p=128),
                              probs_sb[:])

            cT = embT
            for nm in ('c0', 'c1'):
                W, b = tower[nm]
                ps = ptw.tile([128, 512], F32, tag='ptw')
                nc.tensor.matmul(ps[:], W[:], cT[:].bitcast(W.dtype), start=True, stop=True)
                cT = work.tile([128, 512], w_dt, tag=nm)
                nc.scalar.activation(cT[:], ps[:], AF.Relu, bias=b[:], scale=1.0)
            ps_c = ptw.tile([1, 512], F32, tag='ptw')
            nc.tensor.matmul(ps_c[:], Wc2[:], cT[:].bitcast(Wc2.dtype), start=True, stop=True)
            crit = work.tile([1, 512], F32, tag='crit')
            nc.scalar.activation(crit[:], ps_c[:], AF.Identity, bias=bc2[:], scale=1.0)
            nc.sync.dma_start(critic_d[rows], crit[:])

        nc.sync.dma_start(accm_d[:], acc_m[:])
        nc.sync.dma_start(accq_d[:], acc_q[:])

    nc.compile()
    _split_excess_waits(nc)
    return nc


def host_inputs(inp, R, core):
    x = np.asarray(inp['x']).reshape(-1, 1024)
    E = np.asarray(inp['E'])
    E2 = np.concatenate([2.0 * E.T, -(E * E).sum(1)[None, :]], 0).astype(np.float32)
    return {
        'x': np.ascontiguousarray(x[core * R:(core + 1) * R]),
        'We0': np.asarray(inp['W_e0']), 'be0': np.asarray(inp['b_e0']),
        'We1': np.asarray(inp['W_e1']), 'be1': np.asarray(inp['b_e1']),
        'We2': np.asarray(inp['W_e2']), 'be2': np.asarray(inp['b_e2']),
        'Wp': np.asarray(inp['W_p']), 'bp': np.asarray(inp['b_p']),
        'E2': E2,
        'Wa0': np.asarray(inp['W_a0']), 'ba0': np.asarray(inp['b_a0']),
        'Wa1': np.asarray(inp['W_a1']),
        'Wc0': np.asarray(inp['W_c0']), 'bc0': np.asarray(inp['b_c0']),
        'Wc1': np.asarray(inp['W_c1']), 'bc1': np.asarray(inp['b_c1']),
        'Wc2': np.asarray(inp['W_c2']), 'bc2': np.asarray(inp['b_c2']),
    }


_CACHE = {}


def _get_nc(R, cfg_key):
    key = (R, cfg_key)
    if key not in _CACHE:
        _CACHE[key] = build(R, CFG)
    return _CACHE[key]


def kernel(**inputs):
    B = np.asarray(inputs['x']).shape[0]
    n_cores = 8
    R = B // n_cores
    assert R * n_cores == B
    nc = _get_nc(R, tuple(sorted(CFG.items())))
    in_maps = [host_inputs(inputs, R, c) for c in range(n_cores)]
    res = run_bass_kernel_spmd(nc, in_maps, list(range(n_cores))).results

    probs = np.concatenate([r['probs'] for r in res], 0)
    critic = np.concatenate([r['critic'] for r in res], 0)
    idx = np.concatenate([r['idx'] for r in res], 0).astype(np.int32)
    tot = sum(float(r['accq'].sum() - r['accm'].sum()) for r in res)
    vq_loss = np.float32(1.25 * tot / (B * 64))

    ba1 = np.asarray(inputs['b_a1'])
    if np.any(ba1 != 0):
        # kernel computes softmax of (a @ W_a1); fold the bias exactly:
        # softmax(z + b) = normalize(softmax(z) * exp(b))
        w = probs * np.exp(ba1)[None, :]
        probs = (w / w.sum(1, keepdims=True)).astype(np.float32)

    return probs, critic, vq_loss, idx


# revision 2
# speedup vs baseline: 1.5452x; 1.5452x over previous
"""Trainium2 Bass kernel for nn_ActorCriticSpeakerDenseQuantized.

kernel(**inputs) takes the FULL (unsharded) inputs exactly as produced by
setup_inputs() and returns the full outputs as the reference does:
(probs [65536,128] f32, critic [65536] f32, vq_loss scalar f32,
 idx [65536] int32).

Sharding: pure data parallel over the batch across 8 NeuronCores (8192 rows
each); all weights and the 512x64 codebook are replicated. The vq_loss mean
is assembled on the host from per-core partial sums (the psum-mean across
shards).

Per-core pipeline, per 512-row chunk:
  x [512,1024] --PE transpose (fp32, exact)--> xfT (features on partitions),
    rounded to float32r at the PSUM->SBUF copy (DVE/ACT alternating).
  The embedding tower / W_p projection / heads run as lhsT.T @ rhs matmuls
  with rows on the moving free dim (N=512) in float32r (full PE rate);
  relu+bias fused on the scalar engine.
  VQ: u[r,k] = 2*xq_r.E_k - ||E_k||^2 via a K=65 matmul (a ones row folds the
  codebook norms and b_p). To keep fp32-class precision at float32r speed the
  product is computed with a residual split (xq = xq_r + xq_lo,
  E2 = E2_r + E2_lo, u ~= xq_r.E2_r + xq_r.E2_lo + xq_lo.E2_r).
  argmin d == argmax u via the DVE max8/max_index instructions straight out
  of PSUM (first-occurrence tie semantics match jnp.argmin).
  d_min = ||xq||^2 - max_u accumulated per partition for the loss
  (sum xq^2 rides the scalar-engine Square accum_out for free).
  actor head: logits per 128-row group in [rows, A] layout; softmax as exp
  (ACT, accumulated row sum; logits are bounded ~ +-3 so no max subtraction)
  + normalize_recip on GPSIMD.
  critic head: c2 as an M=1 matmul -> [1, rows].
"""
import sys

sys.path.insert(0, '/opt/trn_rl_repo')

import numpy as np
from contextlib import ExitStack

import concourse.bass as bass
import concourse.tile as tile
from concourse import bacc, masks, mybir
from concourse.vector_clock import ScopedClock
from concourse.bass_utils import run_bass_kernel_spmd

F32 = mybir.dt.float32
F32R = mybir.dt.float32r
AF = mybir.ActivationFunctionType

MAX_WAITS = 1

# 'f32'/'f32'    : every matmul full fp32 (max precision, ~1.6x slower)
# 'f32r'/'f32r_res': float32r matmuls, VQ distances residual-corrected
CFG = {'mm': 'f32r', 'vq': 'f32r_res'}

N_CORES = 8


def _split_excess_waits(nc, maxw=MAX_WAITS):
    """walrus in this toolchain rejects instructions carrying more than one
    semaphore wait. Hoist excess waits onto same-engine NoOp carriers placed
    immediately before the offending instruction (same per-engine program
    order => identical semantics)."""
    for f in nc.m.functions:
        for b in f.blocks:
            il = list(b.instructions)
            out = []
            changed = False
            for inst in il:
                si = getattr(inst, 'sync_info', None)
                waits = list(si.on_wait) if si is not None and si.on_wait else []
                if len(waits) > maxw:
                    changed = True
                    extra, keep = waits[:-maxw], waits[-maxw:]
                    for i in range(0, len(extra), maxw):
                        nop = mybir.InstNoOp(name=f"WSPLIT-{nc.next_id()}",
                                             ins=[], outs=[])
                        nop.engine = inst.engine
                        nop.sync_info = mybir.SyncInfo(on_wait=extra[i:i + maxw],
                                                       on_update=[])
                        out.append(nop)
                    si.on_wait = keep
                out.append(inst)
            if changed:
                b.instructions = out


class TC(tile.TileContext):
    """TileContext with the tail drain left multi-wait; _split_excess_waits
    runs after Bacc compile and splits every over-limit instruction."""

    def _drain_and_barrier(self, tick_clock, wait_clock):
        drain_inst = self.nc.sync.drain()
        wait_clock.add_sem_waits(drain_inst.ins,
                                 ScopedClock({None: tick_clock.global_clock}))
        self.nc.all_engine_barrier()
        popped = self.nc._tile_sem_poison_stack.pop()
        assert popped is self._sem_poison
        self.nc.clear_and_free_semaphores(list(self.sems.allocated().values()))
        self.nc.all_engine_barrier()


def build(R, cfg=None, loop=1):
    cfg = cfg or {}
    mm_dt = cfg.get('mm', 'f32')
    vq_mode = cfg.get('vq', 'f32')
    assert (mm_dt, vq_mode) in (('f32', 'f32'), ('f32r', 'f32r_res'))
    assert R % 512 == 0
    NCH = R // 512
    F, H, DV, K, A = 1024, 128, 64, 512, 128

    nc = bacc.Bacc()

    x_d = nc.declare_dram_parameter('x', [R, F], F32, isOutput=False)
    We0_d = nc.declare_dram_parameter('We0', [F, H], F32, isOutput=False)
    be0_d = nc.declare_dram_parameter('be0', [H], F32, isOutput=False)
    We1_d = nc.declare_dram_parameter('We1', [H, H], F32, isOutput=False)
    be1_d = nc.declare_dram_parameter('be1', [H], F32, isOutput=False)
    We2_d = nc.declare_dram_parameter('We2', [H, H], F32, isOutput=False)
    be2_d = nc.declare_dram_parameter('be2', [H], F32, isOutput=False)
    Wp_d = nc.declare_dram_parameter('Wp', [F, DV], F32, isOutput=False)
    bp_d = nc.declare_dram_parameter('bp', [DV], F32, isOutput=False)
    E2_d = nc.declare_dram_parameter('E2', [DV + 1, K], F32, isOutput=False)
    Wa0_d = nc.declare_dram_parameter('Wa0', [H, H], F32, isOutput=False)
    ba0_d = nc.declare_dram_parameter('ba0', [H], F32, isOutput=False)
    Wa1_d = nc.declare_dram_parameter('Wa1', [H, A], F32, isOutput=False)
    Wc0_d = nc.declare_dram_parameter('Wc0', [H, H], F32, isOutput=False)
    bc0_d = nc.declare_dram_parameter('bc0', [H], F32, isOutput=False)
    Wc1_d = nc.declare_dram_parameter('Wc1', [H, H], F32, isOutput=False)
    bc1_d = nc.declare_dram_parameter('bc1', [H], F32, isOutput=False)
    Wc2_d = nc.declare_dram_parameter('Wc2', [H, 1], F32, isOutput=False)
    bc2_d = nc.declare_dram_parameter('bc2', [1], F32, isOutput=False)

    probs_d = nc.declare_dram_parameter('probs', [R, A], F32, isOutput=True)
    critic_d = nc.declare_dram_parameter('critic', [R], F32, isOutput=True)
    idx_d = nc.declare_dram_parameter('idx', [R], mybir.dt.int32, isOutput=True)
    accm_d = nc.declare_dram_parameter('accm', [128, 1], F32, isOutput=True)
    accq_d = nc.declare_dram_parameter('accq', [DV, 1], F32, isOutput=True)

    w_dt = F32 if mm_dt == 'f32' else F32R

    with TC(nc) as tc, ExitStack() as ctx:
        consts = ctx.enter_context(tc.tile_pool(name='consts', bufs=1))
        xin = ctx.enter_context(tc.tile_pool(name='xin', bufs=2))
        xft = ctx.enter_context(tc.tile_pool(name='xft', bufs=2))
        work = ctx.enter_context(tc.tile_pool(name='work', bufs=2))
        accp = ctx.enter_context(tc.tile_pool(name='accp', bufs=1))

        pt = ctx.enter_context(tc.tile_pool(name='pt', bufs=2, space='PSUM'))
        pe0 = ctx.enter_context(tc.tile_pool(name='pe0', bufs=1, space='PSUM'))
        pp = ctx.enter_context(tc.tile_pool(name='pp', bufs=1, space='PSUM'))
        pu = ctx.enter_context(tc.tile_pool(name='pu', bufs=2, space='PSUM'))
        ptw = ctx.enter_context(tc.tile_pool(name='ptw', bufs=2, space='PSUM'))

        ident = consts.tile([128, 128], F32)
        masks.make_identity(nc, ident[:])

        def load_const(shape, dram_ap, dt_out, tag):
            # DMA always as plain fp32; round to float32r with a one-time DVE
            # copy (float32r-typed DMAs mis-lower for some access patterns).
            tf = consts.tile(shape, F32, tag=tag + '_f')
            nc.sync.dma_start(tf[:], dram_ap)
            if dt_out == F32:
                return tf
            tr_ = consts.tile(shape, F32R, tag=tag + '_r')
            nc.vector.tensor_copy(tr_[:], tf[:])
            return tr_

        We0 = load_const([128, 8 * H], We0_d[:].rearrange("(c p) m -> p c m", p=128),
                         w_dt, 'We0')
        pdt = w_dt if vq_mode != 'f32' else F32
        Wp = load_const([128, 8 * DV], Wp_d[:].rearrange("(c p) m -> p c m", p=128),
                        pdt, 'Wp')
        if vq_mode == 'f32r_res':
            E2f = consts.tile([DV + 1, K], F32)
            nc.sync.dma_start(E2f[:], E2_d[:])
            E2r = consts.tile([DV + 1, K], F32R)
            nc.vector.tensor_copy(E2r[:], E2f[:])
            E2lo = consts.tile([DV + 1, K], F32R)
            nc.vector.tensor_sub(E2lo[:], E2f[:], E2r[:].bitcast(F32))
        else:
            E2 = consts.tile([DV + 1, K], F32)
            nc.sync.dma_start(E2[:], E2_d[:])
        tower = {}
        for nm, wd, bd in (('e1', We1_d, be1_d), ('e2', We2_d, be2_d),
                           ('a0', Wa0_d, ba0_d), ('c0', Wc0_d, bc0_d),
                           ('c1', Wc1_d, bc1_d)):
            W = load_const([128, 128], wd[:], w_dt, f'W_{nm}')
            b = consts.tile([128, 1], F32, tag=f'b_{nm}')
            nc.sync.dma_start(b[:], bd[:].unsqueeze(1))
            tower[nm] = (W, b)
        Wa1 = load_const([128, A], Wa1_d[:], w_dt, 'Wa1')
        Wc2 = load_const([128, 1], Wc2_d[:], w_dt, 'Wc2')
        be0 = consts.tile([128, 1], F32)
        nc.sync.dma_start(be0[:], be0_d[:].unsqueeze(1))
        bp = consts.tile([DV, 1], F32)
        nc.sync.dma_start(bp[:], bp_d[:].unsqueeze(1))
        bc2 = consts.tile([1, 1], F32)
        nc.sync.dma_start(bc2[:], bc2_d[:].unsqueeze(1))

        acc_m = accp.tile([128, 1], F32)
        acc_q = accp.tile([DV, 1], F32)
        nc.gpsimd.memset(acc_m[:], 0.0)
        nc.gpsimd.memset(acc_q[:], 0.0)

        lctx = tc.For_i(0, loop, 1) if loop > 1 else None
        if lctx is not None:
            lctx.__enter__()
        for c in range(NCH):
            rows = slice(c * 512, (c + 1) * 512)
            x_sb = xin.tile([128, 4096], F32, tag='x')
            nc.sync.dma_start(x_sb[:],
                              x_d[rows, :].rearrange("(g p) f -> p g f", p=128))
            xfT = xft.tile([128, 4096], pdt, tag='xfT')
            for fc in range(8):
                ps_t = pt.tile([128, 512], F32, tag='pt')
                for g in range(4):
                    nc.tensor.transpose(
                        ps_t[:, g * 128:(g + 1) * 128],
                        x_sb[:, g * 1024 + fc * 128: g * 1024 + (fc + 1) * 128],
                        ident[:])
                if fc % 2 == 0:
                    nc.vector.tensor_copy(xfT[:, fc * 512:(fc + 1) * 512], ps_t[:])
                else:
                    nc.scalar.copy(xfT[:, fc * 512:(fc + 1) * 512], ps_t[:])

            # ---- e0 ----
            ps_e = pe0.tile([128, 512], F32, tag='pe')
            for fc in range(8):
                nc.tensor.matmul(ps_e[:], We0[:, fc * 128:(fc + 1) * 128],
                                 xfT[:, fc * 512:(fc + 1) * 512].bitcast(We0.dtype),
                                 start=(fc == 0), stop=(fc == 7))
            embT = work.tile([128, 512], w_dt, tag='emb1')
            nc.scalar.activation(embT[:], ps_e[:], AF.Relu, bias=be0[:], scale=1.0)

            # ---- p (xq) ----
            ps_p = pp.tile([DV, 512], F32, tag='pp')
            for fc in range(8):
                nc.tensor.matmul(ps_p[:], Wp[:, fc * DV:(fc + 1) * DV],
                                 xfT[:, fc * 512:(fc + 1) * 512],
                                 start=(fc == 0), stop=(fc == 7))
            xq_aug = work.tile([DV + 1, 512], F32, tag='xq')
            nc.gpsimd.memset(xq_aug[DV:DV + 1, :], 1.0)
            xq_f32_view = xq_aug[0:DV, :].bitcast(F32)
            nc.scalar.activation(xq_f32_view, ps_p[:], AF.Identity,
                                 bias=bp[:], scale=1.0)
            if vq_mode == 'f32r_res':
                xq_r = work.tile([DV + 1, 512], F32R, tag='xqr')
                nc.vector.tensor_copy(xq_r[:], xq_aug[:])
                xq_lo = work.tile([DV + 1, 512], F32R, tag='xqlo')
                nc.vector.tensor_sub(xq_lo[:], xq_aug[:], xq_r[:].bitcast(F32))

            # sum of xq^2 for the loss (free accumulation on ACT)
            sqs = work.tile([DV, 1], F32, tag='sqs')
            sq_scr = work.tile([DV, 512], F32, tag='sqscr')
            nc.scalar.activation(sq_scr[:], xq_f32_view, AF.Square,
                                 bias=0.0, scale=1.0, accum_out=sqs[:])
            nc.vector.tensor_add(acc_q[:], acc_q[:], sqs[:])

            # ---- tower e1, e2 ----
            for nm in ('e1', 'e2'):
                W, b = tower[nm]
                ps = ptw.tile([128, 512], F32, tag='ptw')
                nc.tensor.matmul(ps[:], W[:], embT[:].bitcast(W.dtype),
                                 start=True, stop=True)
                embT = work.tile([128, 512], w_dt, tag=nm)
                nc.scalar.activation(embT[:], ps[:], AF.Relu, bias=b[:], scale=1.0)

            # ---- VQ ----
            idx_sb = work.tile([128, 4], mybir.dt.int32, tag='idxc')
            for g in range(4):
                ps_u = pu.tile([128, 512], F32, tag='pu')
                if vq_mode == 'f32r_res':
                    nc.tensor.matmul(ps_u[:], xq_r[:, g * 128:(g + 1) * 128],
                                     E2r[:], start=True, stop=False)
                    nc.tensor.matmul(ps_u[:], xq_r[:, g * 128:(g + 1) * 128],
                                     E2lo[:], start=False, stop=False)
                    nc.tensor.matmul(ps_u[:], xq_lo[:, g * 128:(g + 1) * 128],
                                     E2r[:], start=False, stop=True)
                else:
                    nc.tensor.matmul(ps_u[:], xq_aug[:, g * 128:(g + 1) * 128],
                                     E2[:], start=True, stop=True)
                mx = work.tile([128, 8], F32, tag='mx')
                mi = work.tile([128, 8], mybir.dt.uint32, tag='mi')
                nc.vector.max(mx[:], ps_u[:])
                nc.vector.max_index(mi[:], mx[:], ps_u[:])
                nc.vector.tensor_add(acc_m[:], acc_m[:], mx[:, 0:1])
                nc.vector.tensor_copy(idx_sb[:, g:g + 1], mi[:, 0:1])
            nc.sync.dma_start(idx_d[rows].rearrange("(g p) -> p g", p=128), idx_sb[:])

            # ---- actor ----
            W, b = tower['a0']
            ps = ptw.tile([128, 512], F32, tag='ptw')
            nc.tensor.matmul(ps[:], W[:], embT[:].bitcast(W.dtype),
                             start=True, stop=True)
            aT = work.tile([128, 512], w_dt, tag='aT')
            nc.scalar.activation(aT[:], ps[:], AF.Relu, bias=b[:], scale=1.0)

            probs_sb = work.tile([128, 512], F32, tag='probs')
            ps_l = ptw.tile([128, 512], F32, tag='ptw')
            for g in range(4):
                nc.tensor.matmul(ps_l[:, g * 128:(g + 1) * 128],
                                 aT[:, g * 128:(g + 1) * 128], Wa1[:],
                                 start=True, stop=True)
            for g in range(4):
                ex = work.tile([128, 128], F32, tag='exp')
                den = work.tile([128, 1], F32, tag='den')
                nc.scalar.activation(ex[:], ps_l[:, g * 128:(g + 1) * 128], AF.Exp,
                                     bias=0.0, scale=1.0, accum_out=den[:])
                nc.gpsimd.normalize_recip(probs_sb[:, g * 128:(g + 1) * 128],
                                          ex[:], den[:])
            nc.sync.dma_start(probs_d[rows, :].rearrange("(g p) a -> p g a", p=128),
                              probs_sb[:])

            # ---- critic ----
            cT = embT
            for nm in ('c0', 'c1'):
                W, b = tower[nm]
                ps = ptw.tile([128, 512], F32, tag='ptw')
                nc.tensor.matmul(ps[:], W[:], cT[:].bitcast(W.dtype),
                                 start=True, stop=True)
                cT = work.tile([128, 512], w_dt, tag=nm)
                nc.scalar.activation(cT[:], ps[:], AF.Relu, bias=b[:], scale=1.0)
            ps_c = ptw.tile([1, 512], F32, tag='ptw')
            nc.tensor.matmul(ps_c[:], Wc2[:], cT[:].bitcast(Wc2.dtype),
                             start=True, stop=True)
            crit = work.tile([1, 512], F32, tag='crit')
            nc.scalar.activation(crit[:], ps_c[:], AF.Identity, bias=bc2[:], scale=1.0)
            nc.sync.dma_start(critic_d[rows], crit[:])

        if lctx is not None:
            lctx.__exit__(None, None, None)
        nc.sync.dma_start(accm_d[:], acc_m[:])
        nc.sync.dma_start(accq_d[:], acc_q[:])

    nc.compile()
    _split_excess_waits(nc)
    return nc


def host_inputs(inp, R, core):
    x = np.asarray(inp['x']).reshape(-1, 1024)
    E = np.asarray(inp['E'])
    E2 = np.concatenate([2.0 * E.T, -(E * E).sum(1)[None, :]], 0).astype(np.float32)
    return {
        'x': np.ascontiguousarray(x[core * R:(core + 1) * R]),
        'We0': np.asarray(inp['W_e0']), 'be0': np.asarray(inp['b_e0']),
        'We1': np.asarray(inp['W_e1']), 'be1': np.asarray(inp['b_e1']),
        'We2': np.asarray(inp['W_e2']), 'be2': np.asarray(inp['b_e2']),
        'Wp': np.asarray(inp['W_p']), 'bp': np.asarray(inp['b_p']),
        'E2': E2,
        'Wa0': np.asarray(inp['W_a0']), 'ba0': np.asarray(inp['b_a0']),
        'Wa1': np.asarray(inp['W_a1']),
        'Wc0': np.asarray(inp['W_c0']), 'bc0': np.asarray(inp['b_c0']),
        'Wc1': np.asarray(inp['W_c1']), 'bc1': np.asarray(inp['b_c1']),
        'Wc2': np.asarray(inp['W_c2']), 'bc2': np.asarray(inp['b_c2']),
    }


_CACHE = {}


def get_nc(R, cfg=None, loop=1):
    cfg = dict(CFG if cfg is None else cfg)
    key = (R, tuple(sorted(cfg.items())), loop)
    if key not in _CACHE:
        _CACHE[key] = build(R, cfg, loop=loop)
    return _CACHE[key]


def kernel(**inputs):
    x = np.asarray(inputs['x'])
    B = x.shape[0]
    R = B // N_CORES
    assert R * N_CORES == B and R % 512 == 0
    nc = get_nc(R)
    in_maps = [host_inputs(inputs, R, c) for c in range(N_CORES)]
    res = run_bass_kernel_spmd(nc, in_maps, list(range(N_CORES))).results

    probs = np.concatenate([r['probs'] for r in res], 0)
    critic = np.concatenate([r['critic'] for r in res], 0)
    idx = np.concatenate([r['idx'] for r in res], 0).astype(np.int32)
    tot = sum(float(r['accq'].sum() - r['accm'].sum()) for r in res)
    vq_loss = np.float32(1.25 * tot / (B * 64))

    ba1 = np.asarray(inputs['b_a1'])
    if np.any(ba1 != 0):
        # device computes softmax of (a @ W_a1); fold the bias exactly:
        # softmax(z + b) = normalize(softmax(z) * exp(b))
        w = probs * np.exp(ba1)[None, :].astype(np.float32)
        probs = (w / w.sum(1, keepdims=True)).astype(np.float32)

    return probs, critic, vq_loss, idx
